# revision 16
# baseline (speedup 1.0000x reference)
"""DualMem retrieval kernel for Trainium2 (8 NeuronCores, Bass/Tile).

Math (per reference):
    sim[b,c,m]  = <img[b], mem[c,m]>
    w           = exp(-beta * (1 - sim))
    adapt[b,c]  = sum_m mem[c,m] * w[b,c,m]
    logits[b,c] = 100 * <img[b], adapt[b,c] / ||adapt[b,c]||>

Key algebraic reduction (avoids materializing adapt [B,C,D]):
    numer[b,c]  = <img[b], adapt[b,c]> = sum_m w[b,c,m] * sim[b,c,m]
    denom[b,c]  = ||adapt[b,c]||^2     = w^T G_c w,  G_c = mem_c @ mem_c^T  (11x11 Gram)
    logits      = 100 * numer / sqrt(denom)

Sharding: classes C=1000 split 125 per core across 8 cores (mem bank fully
sharded; only img replicated).

Per-core layout: groups of 11 classes x 11 memories = 121 partitions (pad to
128); 12 groups cover 132 >= 125 class slots.  The bf16 memory bank arrives
class-major and is transposed on-chip (HW xbar DMA transpose) to [d, cm].
For each group g and d-chunk i (D=1024 = 8*128), with stationary memT block:
    sim tile [cm, b]  = sum_i memT_gi.T @ imgT_i      (PSUM cols 0:64)
    G tile  [cm, cm'] = sum_i memT_gi.T @ memT_gi     (separate PSUM tile)
Then w=exp(beta*sim-beta) (ACT), G masked to class-diagonal blocks (DVE),
u = G_m.T @ w lands next to sim in the same PSUM tile so one mul builds
[w*sim | w*u], and an 0/1 "E" matmul sums over m within each class giving
[numer | denom] per class.  Final sqrt/recip/mul batched over all groups.
"""

import os
import sys

sys.path.insert(0, "/opt/trn_rl_repo")

import ml_dtypes
import numpy as np

B, C, M, D = 64, 1000, 11, 1024
BETA = 5.5
N_CORES = 8
C_PER = C // N_CORES          # 125 classes per core
CPG = 11                      # classes per group
NG = 12                       # groups per core (132 class slots >= 125)
PG = CPG * M                  # 121 used partitions per group
DCH = D // 128                # 8 d-chunks
ROWS = NG * 128               # 1536 class-major rows per core
NDB = 4                       # groups per E-matmul block (rhs 512 wide)

_cache = {}


def _build():
    import concourse.mybir as mybir
    import concourse.tile as tile
    from concourse import bacc

    f32 = mybir.dt.float32
    bf16 = mybir.dt.bfloat16

    nc = bacc.Bacc("TRN2", target_bir_lowering=False, debug=False,
                   num_devices=N_CORES)

    membf = nc.dram_tensor("membf", [ROWS, D], bf16, kind="ExternalInput")
    imgt = nc.dram_tensor("imgt", [128, DCH, 64], bf16, kind="ExternalInput")
    mask = nc.dram_tensor("mask", [128, 128], f32, kind="ExternalInput")
    em = nc.dram_tensor("em", [128, 16], f32, kind="ExternalInput")
    out = nc.dram_tensor("out", [16, NG * 64], f32, kind="ExternalOutput")

    with tile.TileContext(nc) as tc:
        with (
            tc.tile_pool(name="const", bufs=1) as const,
            tc.tile_pool(name="sb", bufs=3) as sb,
            tc.tile_pool(name="ps_su", bufs=3, space="PSUM") as ps_su,
            tc.tile_pool(name="ps_g", bufs=3, space="PSUM") as ps_g,
            tc.tile_pool(name="ps_nd", bufs=2, space="PSUM") as ps_nd,
        ):
            # memT[d % 128, d_chunk, cm], one tile per quarter-bank so the
            # xbar transposes and the matmuls reading them don't false-share
            Q = 4
            H = ROWS // Q
            GPQ = NG // Q                  # groups per quarter
            mt = [const.tile([128, DCH, H], bf16, name=f"mt{q}", tag=f"mt{q}")
                  for q in range(Q)]
            it = const.tile([128, DCH, 64], bf16)
            mask_sb = const.tile([128, 128], f32)
            em_sb = const.tile([128, 16], f32)
            wsq = [const.tile([128, NDB * 128], f32, name=f"wsq{nb}",
                              tag=f"wsq{nb}") for nb in range(NG // NDB)]
            ndall = const.tile([16, NG * 128], f32)
            lg = const.tile([16, NG * 64], f32)
            bias_exp = const.tile([128, 1], f32)
            bias_eps = const.tile([16, 1], f32)
            nc.vector.memset(bias_exp[:], -BETA)
            nc.vector.memset(bias_eps[:], 1e-30)

            # DMA-transpose the class-major bank into [d, cm].  One xbar
            # transpose per quarter-bank: out[p, i, r] = membf[r, 128*i + p].
            # First quarter goes ahead of the small const loads so group-0
            # compute starts as early as possible.
            # const copies ride SWDGE (gpsimd) so the sync queue is free to
            # start the xbar transposes immediately
            nc.gpsimd.dma_start(it[:], imgt.ap())
            nc.gpsimd.dma_start(mask_sb[:], mask.ap())
            nc.gpsimd.dma_start(em_sb[:], em.ap())
            for q in range(Q):
                nc.sync.dma_start(
                    mt[q][:],
                    membf.ap()[q * H:(q + 1) * H, :],
                    transpose=True,
                )

            for g in range(NG):
                # su[:, 0:64] accumulates sim over d-chunks; 64:128 gets u later
                su = ps_su.tile([128, 128], f32)
                gm_ps = ps_g.tile([128, 128], f32)
                for i in range(DCH):
                    blk = mt[g // GPQ][:, i, (g % GPQ) * 128:(g % GPQ + 1) * 128]
                    nc.tensor.matmul(su[:, 0:64], blk, it[:, i, :],
                                     start=(i == 0), stop=(i == DCH - 1))
                    nc.tensor.matmul(gm_ps[:], blk, blk,
                                     start=(i == 0), stop=(i == DCH - 1))

                # w = exp(beta*sim - beta)
                w = sb.tile([128, 64], f32, tag="w")
                nc.scalar.activation(w[:], su[:, 0:64],
                                     mybir.ActivationFunctionType.Exp,
                                     bias=bias_exp[:], scale=BETA)

                # masked Gram -> SBUF (kills cross-class + pad entries)
                gm = sb.tile([128, 128], f32, tag="gm")
                nc.vector.tensor_mul(gm[:], gm_ps[:], mask_sb[:])

                # u = G_masked^T @ w, placed next to sim in the same bank
                nc.tensor.matmul(su[:, 64:128], gm[:], w[:],
                                 start=True, stop=True)

                # [w*sim | w*u]
                ws = wsq[g // NDB][:, (g % NDB) * 128:(g % NDB + 1) * 128]
                nc.vector.tensor_mul(ws[0:128, 0:64], su[:, 0:64], w[:])
                nc.vector.tensor_mul(ws[0:128, 64:128], su[:, 64:128], w[:])

            # E-reduction: nd[c, :] = [numer | denom] per class, 4 groups/shot
            for nb in range(NG // NDB):
                nd = ps_nd.tile([16, NDB * 128], f32)
                nc.tensor.matmul(nd[:], em_sb[:], wsq[nb][:],
                                 start=True, stop=True)
                nc.vector.tensor_copy(
                    ndall[:, nb * NDB * 128:(nb + 1) * NDB * 128], nd[:])

            # logits = numer * 100/sqrt(denom), batched over all groups
            nd3 = ndall[:].rearrange("p (g t b) -> p g t b", g=NG, t=2)
            s_all = sb.tile([16, NG * 64], f32, tag="s")
            nc.scalar.activation(s_all[:], nd3[:, :, 1, :],
                                 mybir.ActivationFunctionType.Sqrt,
                                 bias=bias_eps[:], scale=1e-4)
            r_all = sb.tile([16, NG * 64], f32, tag="r")
            nc.vector.reciprocal(r_all[:], s_all[:])
            nc.vector.tensor_mul(lg[:], nd3[:, :, 0, :], r_all[:])

            nc.sync.dma_start(out.ap(), lg[:])

    nc.compile()
    return nc


def _get_nc():
    if "nc" not in _cache:
        _cache["nc"] = _build()
    return _cache["nc"]


def _prep_inputs(img_features, memorized_image_feat):
    """Host-side formatting: bf16 cast, class padding, group layout."""
    bf = ml_dtypes.bfloat16
    img_b = np.ascontiguousarray(img_features.astype(bf))          # [64, 1024]
    mem_b = memorized_image_feat.astype(bf)                        # [1000,11,1024]

    # imgt[p, i, b] = img[b, i*128+p]
    imgt = np.ascontiguousarray(
        img_b.T.reshape(DCH, 128, 64).transpose(1, 0, 2))          # [128, 8, 64]

    mask = np.zeros((128, 128), np.float32)
    for c in range(CPG):
        mask[c * M:(c + 1) * M, c * M:(c + 1) * M] = 1.0
    em = np.zeros((128, 16), np.float32)
    for c in range(CPG):
        em[c * M:(c + 1) * M, c] = 1.0

    in_maps = []
    for k in range(N_CORES):
        sl = mem_b[k * C_PER:(k + 1) * C_PER]                      # [125,11,1024]
        pad = np.zeros((NG * CPG, M, D), bf)
        pad[:C_PER] = sl
        grp = pad.reshape(NG, PG, D)
        full = np.zeros((NG, 128, D), bf)
        full[:, :PG] = grp
        membf = np.ascontiguousarray(full.reshape(ROWS, D))
        in_maps.append({"membf": membf, "imgt": imgt,
                        "mask": mask, "em": em})
    return in_maps


def _gather(results):
    logits = np.empty((B, C), np.float32)
    for k in range(N_CORES):
        o = results[k]["out"].reshape(16, NG, 64)[:CPG]            # [11, 12, 64]
        o = o.transpose(1, 0, 2).reshape(NG * CPG, 64)[:C_PER]     # [125, 64]
        logits[:, k * C_PER:(k + 1) * C_PER] = o.T
    return logits


def kernel(img_features, memorized_image_feat):
    from concourse.bass_utils import run_bass_kernel_spmd

    nc = _get_nc()
    in_maps = _prep_inputs(img_features, memorized_image_feat)
    res = run_bass_kernel_spmd(nc, in_maps, core_ids=list(range(N_CORES)))
    return _gather(res.results)


# revision 17
# speedup vs baseline: 1.1075x; 1.1075x over previous
"""DualMem retrieval kernel for Trainium2 (8 NeuronCores, Bass/Tile).

Math (per reference):
    sim[b,c,m]  = <img[b], mem[c,m]>
    w           = exp(-beta * (1 - sim))
    adapt[b,c]  = sum_m mem[c,m] * w[b,c,m]
    logits[b,c] = 100 * <img[b], adapt[b,c] / ||adapt[b,c]||>

Key algebraic reduction (avoids materializing adapt [B,C,D]):
    numer[b,c]  = <img[b], adapt[b,c]> = sum_m w[b,c,m] * sim[b,c,m]
    denom[b,c]  = ||adapt[b,c]||^2     = w^T G_c w,  G_c = mem_c @ mem_c^T  (11x11 Gram)
    logits      = 100 * numer / sqrt(denom)

Sharding: classes C=1000 split 125 per core across 8 cores (mem bank fully
sharded; only img replicated).

Per-core layout: groups of 11 classes x 11 memories = 121 partitions (pad to
128); 12 groups cover 132 >= 125 class slots.  The bf16 memory bank arrives
class-major and is xbar-DMA-transposed on-chip to [d, cm].  Groups are
processed in blocks of 4 sharing PSUM banks (per-element has_written makes
disjoint column ranges in one bank legal accumulation groups; the bank-level
software check is skipped):
    su bank  [128, 512]: per group k, cols 128k+0:64  = sim (acc over d)
                                      cols 128k+64:128 = u = G_masked^T w
    G bank   [128, 512]: per group k, cols 128k : 128k+128 = Gram (acc over d)
Downstream per block: one batched exp, one masked-Gram copy, four u-matmuls,
two strided muls building [w*sim | w*u], one 0/1 "E" matmul summing over m
per class -> [numer | denom], then one batched sqrt/recip/mul at the end.
"""

import os
import sys

sys.path.insert(0, "/opt/trn_rl_repo")

import ml_dtypes
import numpy as np

B, C, M, D = 64, 1000, 11, 1024
BETA = 5.5
N_CORES = 8
C_PER = C // N_CORES          # 125 classes per core
CPG = 11                      # classes per group
NG = 12                       # groups per core (132 class slots >= 125)
PG = CPG * M                  # 121 used partitions per group
DCH = D // 128                # 8 d-chunks
ROWS = NG * 128               # 1536 class-major rows per core
GPB = 4                       # groups per PSUM block
NB = NG // GPB                # 3 blocks

_cache = {}


def _build():
    import concourse.mybir as mybir
    import concourse.tile as tile
    from concourse import bacc

    f32 = mybir.dt.float32
    bf16 = mybir.dt.bfloat16

    nc = bacc.Bacc("TRN2", target_bir_lowering=False, debug=False,
                   num_devices=N_CORES)

    membf = nc.dram_tensor("membf", [ROWS, D], bf16, kind="ExternalInput")
    imgt = nc.dram_tensor("imgt", [128, DCH, 64], bf16, kind="ExternalInput")
    mask = nc.dram_tensor("mask", [128, GPB * 128], f32, kind="ExternalInput")
    em = nc.dram_tensor("em", [128, 16], f32, kind="ExternalInput")
    out = nc.dram_tensor("out", [16, NG * 64], f32, kind="ExternalOutput")

    with tile.TileContext(nc) as tc:
        with (
            tc.tile_pool(name="const", bufs=1) as const,
            tc.tile_pool(name="sb", bufs=2) as sb,
            tc.tile_pool(name="ps_su", bufs=2, space="PSUM") as ps_su,
            tc.tile_pool(name="ps_g", bufs=2, space="PSUM") as ps_g,
            tc.tile_pool(name="ps_nd", bufs=2, space="PSUM") as ps_nd,
        ):
            # memT[d % 128, d_chunk, cm]; one tile per block of 4 groups
            mt = [const.tile([128, DCH, GPB * 128], bf16, name=f"mt{q}",
                             tag=f"mt{q}") for q in range(NB)]
            it = const.tile([128, DCH, 64], bf16)
            mask_sb = const.tile([128, GPB * 128], f32)
            em_sb = const.tile([128, 16], f32)
            ndall = const.tile([16, NG * 128], f32)
            lg = const.tile([16, NG * 64], f32)
            bias_exp = const.tile([128, 1], f32)
            bias_eps = const.tile([16, 1], f32)
            nc.vector.memset(bias_exp[:], -BETA)
            nc.vector.memset(bias_eps[:], 1e-30)

            # const copies first (xbar-mode transitions serialize the DMA
            # path, so keep plain copies together ahead of the transposes)
            nc.sync.dma_start(it[:], imgt.ap())
            nc.sync.dma_start(mask_sb[:], mask.ap())
            nc.sync.dma_start(em_sb[:], em.ap())
            H = ROWS // NB
            for q in range(NB):
                nc.sync.dma_start(
                    mt[q][:],
                    membf.ap()[q * H:(q + 1) * H, :],
                    transpose=True,
                )

            for nb in range(NB):
                su = ps_su.tile([128, GPB * 128], f32)
                gp = ps_g.tile([128, GPB * 128], f32)
                for k in range(GPB):
                    for i in range(DCH):
                        blk = mt[nb][:, i, k * 128:(k + 1) * 128]
                        nc.tensor.matmul(su[:, k * 128:k * 128 + 64],
                                         blk, it[:, i, :],
                                         start=(i == 0), stop=(i == DCH - 1),
                                         skip_group_check=True)
                        nc.tensor.matmul(gp[:, k * 128:(k + 1) * 128],
                                         blk, blk,
                                         start=(i == 0), stop=(i == DCH - 1),
                                         skip_group_check=True)

                # w = exp(beta*sim - beta) for all 4 groups at once
                su4 = su[:].rearrange("p (k t b) -> p k t b", k=GPB, t=2)
                w4 = sb.tile([128, GPB * 64], f32, tag="w4")
                nc.scalar.activation(w4[:], su4[:, :, 0, :],
                                     mybir.ActivationFunctionType.Exp,
                                     bias=bias_exp[:], scale=BETA)

                # masked Gram -> SBUF (kills cross-class + pad entries)
                gm4 = sb.tile([128, GPB * 128], f32, tag="gm4")
                nc.vector.tensor_mul(gm4[:], gp[:], mask_sb[:])

                # u_k = G_k^T @ w_k, placed next to sim_k in the same bank
                for k in range(GPB):
                    nc.tensor.matmul(su[:, k * 128 + 64:(k + 1) * 128],
                                     gm4[:, k * 128:(k + 1) * 128],
                                     w4[:, k * 64:(k + 1) * 64],
                                     start=True, stop=True,
                                     skip_group_check=True)

                # wsq = [w*sim | w*u] in group-blocked layout (two strided muls)
                wsq = sb.tile([128, GPB * 128], f32, tag="wsq")
                wq4 = wsq[:].rearrange("p (k t b) -> p k t b", k=GPB, t=2)
                nc.vector.tensor_mul(wq4[:, :, 0, :], su4[:, :, 0, :], w4[:])
                nc.vector.tensor_mul(wq4[:, :, 1, :], su4[:, :, 1, :], w4[:])

                # nd[c, :] = [numer | denom] per class for the whole block
                nd = ps_nd.tile([16, GPB * 128], f32)
                nc.tensor.matmul(nd[:], em_sb[:], wsq[:],
                                 start=True, stop=True)
                nc.vector.tensor_copy(
                    ndall[:, nb * GPB * 128:(nb + 1) * GPB * 128], nd[:])

            # logits = numer * 100/sqrt(denom), batched over all groups
            nd3 = ndall[:].rearrange("p (g t b) -> p g t b", g=NG, t=2)
            s_all = sb.tile([16, NG * 64], f32, tag="s")
            nc.scalar.activation(s_all[:], nd3[:, :, 1, :],
                                 mybir.ActivationFunctionType.Sqrt,
                                 bias=bias_eps[:], scale=1e-4)
            r_all = sb.tile([16, NG * 64], f32, tag="r")
            nc.vector.reciprocal(r_all[:], s_all[:])
            nc.vector.tensor_mul(lg[:], nd3[:, :, 0, :], r_all[:])

            nc.sync.dma_start(out.ap(), lg[:])

    nc.compile()
    return nc


def _get_nc():
    if "nc" not in _cache:
        _cache["nc"] = _build()
    return _cache["nc"]


def _prep_inputs(img_features, memorized_image_feat):
    """Host-side formatting: bf16 cast, class padding, group layout."""
    bf = ml_dtypes.bfloat16
    img_b = np.ascontiguousarray(img_features.astype(bf))          # [64, 1024]
    mem_b = memorized_image_feat.astype(bf)                        # [1000,11,1024]

    # imgt[p, i, b] = img[b, i*128+p]
    imgt = np.ascontiguousarray(
        img_b.T.reshape(DCH, 128, 64).transpose(1, 0, 2))          # [128, 8, 64]

    m1 = np.zeros((128, 128), np.float32)
    for c in range(CPG):
        m1[c * M:(c + 1) * M, c * M:(c + 1) * M] = 1.0
    mask = np.zeros((128, GPB * 128), np.float32)
    for k in range(GPB):
        mask[:, k * 128:(k + 1) * 128] = m1
    em = np.zeros((128, 16), np.float32)
    for c in range(CPG):
        em[c * M:(c + 1) * M, c] = 1.0

    in_maps = []
    for k in range(N_CORES):
        sl = mem_b[k * C_PER:(k + 1) * C_PER]                      # [125,11,1024]
        pad = np.zeros((NG * CPG, M, D), bf)
        pad[:C_PER] = sl
        grp = pad.reshape(NG, PG, D)
        full = np.zeros((NG, 128, D), bf)
        full[:, :PG] = grp
        membf = np.ascontiguousarray(full.reshape(ROWS, D))
        in_maps.append({"membf": membf, "imgt": imgt,
                        "mask": mask, "em": em})
    return in_maps


def _gather(results):
    logits = np.empty((B, C), np.float32)
    for k in range(N_CORES):
        o = results[k]["out"].reshape(16, NG, 64)[:CPG]            # [11, 12, 64]
        o = o.transpose(1, 0, 2).reshape(NG * CPG, 64)[:C_PER]     # [125, 64]
        logits[:, k * C_PER:(k + 1) * C_PER] = o.T
    return logits


def kernel(img_features, memorized_image_feat):
    from concourse.bass_utils import run_bass_kernel_spmd

    nc = _get_nc()
    in_maps = _prep_inputs(img_features, memorized_image_feat)
    res = run_bass_kernel_spmd(nc, in_maps, core_ids=list(range(N_CORES)))
    return _gather(res.results)


# revision 23
# speedup vs baseline: 1.1123x; 1.0043x over previous
"""DualMem retrieval kernel for Trainium2 (8 NeuronCores, Bass/Tile).

Math (per reference):
    sim[b,c,m]  = <img[b], mem[c,m]>
    w           = exp(-beta * (1 - sim))
    adapt[b,c]  = sum_m mem[c,m] * w[b,c,m]
    logits[b,c] = 100 * <img[b], adapt[b,c] / ||adapt[b,c]||>

Key algebraic reduction (avoids materializing adapt [B,C,D]):
    numer[b,c]  = <img[b], adapt[b,c]> = sum_m w[b,c,m] * sim[b,c,m]
    denom[b,c]  = ||adapt[b,c]||^2     = w^T G_c w,  G_c = mem_c @ mem_c^T  (11x11 Gram)
    logits      = 100 * numer / sqrt(denom)

Sharding: classes C=1000 split 125 per core across 8 cores (mem bank fully
sharded; only img replicated).

Per-core layout: groups of 11 classes x 11 memories = 121 partitions (pad to
128); 12 groups cover 132 >= 125 class slots.  The bf16 memory bank arrives
class-major and is xbar-DMA-transposed on-chip to [d, cm].  Groups are
processed in blocks of 4 sharing PSUM banks (per-element has_written makes
disjoint column ranges in one bank legal accumulation groups; the bank-level
software check is skipped):
    su bank  [128, 512]: per group k, cols 128k+0:64  = sim (acc over d)
                                      cols 128k+64:128 = u = G_masked^T w
    G bank   [128, 512]: per group k, cols 128k : 128k+128 = Gram (acc over d)
Downstream per block: one batched exp, one masked-Gram copy, four u-matmuls,
two strided muls building [w*sim | w*u], one 0/1 "E" matmul summing over m
per class -> [numer | denom], then one batched sqrt/recip/mul at the end.
"""

import os
import sys

sys.path.insert(0, "/opt/trn_rl_repo")

import ml_dtypes
import numpy as np

B, C, M, D = 64, 1000, 11, 1024
BETA = 5.5
N_CORES = 8
C_PER = C // N_CORES          # 125 classes per core
CPG = 11                      # classes per group
NG = 12                       # groups per core (132 class slots >= 125)
PG = CPG * M                  # 121 used partitions per group
DCH = D // 128                # 8 d-chunks
ROWS = NG * 128               # 1536 class-major rows per core
GPB = 4                       # groups per PSUM block
NB = NG // GPB                # 3 blocks

_cache = {}


def _build():
    import concourse.mybir as mybir
    import concourse.tile as tile
    from concourse import bacc

    f32 = mybir.dt.float32
    bf16 = mybir.dt.bfloat16

    nc = bacc.Bacc("TRN2", target_bir_lowering=False, debug=False,
                   num_devices=N_CORES)

    # membf rows: [64 img rows | 1536 class-major memory rows]; the xbar
    # transpose of the first 64 rows lands imgT in exactly the layout the
    # sim matmuls want, so img needs no separate load.
    membf = nc.dram_tensor("membf", [64 + ROWS, D], bf16, kind="ExternalInput")
    mask = nc.dram_tensor("mask", [128, GPB * 128], f32, kind="ExternalInput")
    em = nc.dram_tensor("em", [128, 16], f32, kind="ExternalInput")
    out = nc.dram_tensor("out", [16, NG * 64], f32, kind="ExternalOutput")

    with tile.TileContext(nc) as tc:
        with (
            tc.tile_pool(name="const", bufs=1) as const,
            tc.tile_pool(name="sb", bufs=2) as sb,
            tc.tile_pool(name="ps_su", bufs=2, space="PSUM") as ps_su,
            tc.tile_pool(name="ps_g", bufs=2, space="PSUM") as ps_g,
            tc.tile_pool(name="ps_nd", bufs=2, space="PSUM") as ps_nd,
        ):
            # memT[d % 128, d_chunk, cm]; one tile per block of 4 groups.
            # Tile 0 carries 64 extra leading cm-columns holding imgT.
            mt = [const.tile([128, DCH, (64 if q == 0 else 0) + GPB * 128],
                             bf16, name=f"mt{q}", tag=f"mt{q}")
                  for q in range(NB)]
            it = mt[0][:, :, 0:64]
            mask_sb = const.tile([128, GPB * 128], f32)
            em_sb = const.tile([128, 16], f32)
            ndall = const.tile([16, NG * 128], f32)
            lg = const.tile([16, NG * 64], f32)
            bias_exp = const.tile([128, 1], f32)
            bias_eps = const.tile([16, 1], f32)
            nc.vector.memset(bias_exp[:], -BETA)
            nc.vector.memset(bias_eps[:], 1e-30)

            # xbar transposes first so compute starts ASAP; mask/em (needed
            # only mid-pipeline) follow.  Keeping all plain copies after all
            # transposes avoids repeated xbar-mode serialization stalls.
            H = ROWS // NB
            for q in range(NB):
                ext = 64 if q == 0 else 0
                r0 = 0 if q == 0 else 64 + q * H
                nc.sync.dma_start(
                    mt[q][:],
                    membf.ap()[r0:64 + (q + 1) * H, :],
                    transpose=True,
                )
            nc.sync.dma_start(mask_sb[:], mask.ap())
            nc.sync.dma_start(em_sb[:], em.ap())

            for nb in range(NB):
                su = ps_su.tile([128, GPB * 128], f32)
                gp = ps_g.tile([128, GPB * 128], f32)
                ext = 64 if nb == 0 else 0
                for k in range(GPB):
                    for i in range(DCH):
                        blk = mt[nb][:, i, ext + k * 128:ext + (k + 1) * 128]
                        nc.tensor.matmul(su[:, k * 128:k * 128 + 64],
                                         blk, it[:, i, :],
                                         start=(i == 0), stop=(i == DCH - 1),
                                         skip_group_check=True)
                        nc.tensor.matmul(gp[:, k * 128:(k + 1) * 128],
                                         blk, blk,
                                         start=(i == 0), stop=(i == DCH - 1),
                                         skip_group_check=True)

                # w = exp(beta*sim - beta) for all 4 groups at once
                su4 = su[:].rearrange("p (k t b) -> p k t b", k=GPB, t=2)
                w4 = sb.tile([128, GPB * 64], f32, tag="w4")
                nc.scalar.activation(w4[:], su4[:, :, 0, :],
                                     mybir.ActivationFunctionType.Exp,
                                     bias=bias_exp[:], scale=BETA)

                # masked Gram -> SBUF (kills cross-class + pad entries)
                gm4 = sb.tile([128, GPB * 128], f32, tag="gm4")
                nc.vector.tensor_mul(gm4[:], gp[:], mask_sb[:])

                # u_k = G_k^T @ w_k, placed next to sim_k in the same bank
                for k in range(GPB):
                    nc.tensor.matmul(su[:, k * 128 + 64:(k + 1) * 128],
                                     gm4[:, k * 128:(k + 1) * 128],
                                     w4[:, k * 64:(k + 1) * 64],
                                     start=True, stop=True,
                                     skip_group_check=True)

                # wsq = [w*sim | w*u] in group-blocked layout (two strided muls)
                wsq = sb.tile([128, GPB * 128], f32, tag="wsq")
                wq4 = wsq[:].rearrange("p (k t b) -> p k t b", k=GPB, t=2)
                nc.vector.tensor_mul(wq4[:, :, 0, :], su4[:, :, 0, :], w4[:])
                nc.vector.tensor_mul(wq4[:, :, 1, :], su4[:, :, 1, :], w4[:])

                # nd[c, :] = [numer | denom] per class for the whole block
                nd = ps_nd.tile([16, GPB * 128], f32)
                nc.tensor.matmul(nd[:], em_sb[:], wsq[:],
                                 start=True, stop=True)
                nc.vector.tensor_copy(
                    ndall[:, nb * GPB * 128:(nb + 1) * GPB * 128], nd[:])

            # logits = numer * 100/sqrt(denom), batched over all groups
            nd3 = ndall[:].rearrange("p (g t b) -> p g t b", g=NG, t=2)
            s_all = sb.tile([16, NG * 64], f32, tag="s")
            nc.scalar.activation(s_all[:], nd3[:, :, 1, :],
                                 mybir.ActivationFunctionType.Sqrt,
                                 bias=bias_eps[:], scale=1e-4)
            r_all = sb.tile([16, NG * 64], f32, tag="r")
            nc.vector.reciprocal(r_all[:], s_all[:])
            nc.vector.tensor_mul(lg[:], nd3[:, :, 0, :], r_all[:])

            nc.sync.dma_start(out.ap(), lg[:])

    nc.compile()
    return nc


def _get_nc():
    if "nc" not in _cache:
        _cache["nc"] = _build()
    return _cache["nc"]


def _prep_inputs(img_features, memorized_image_feat):
    """Host-side formatting: bf16 cast, class padding, group layout."""
    bf = ml_dtypes.bfloat16
    img_b = np.ascontiguousarray(img_features.astype(bf))          # [64, 1024]
    mem_b = memorized_image_feat.astype(bf)                        # [1000,11,1024]

    m1 = np.zeros((128, 128), np.float32)
    for c in range(CPG):
        m1[c * M:(c + 1) * M, c * M:(c + 1) * M] = 1.0
    mask = np.zeros((128, GPB * 128), np.float32)
    for k in range(GPB):
        mask[:, k * 128:(k + 1) * 128] = m1
    em = np.zeros((128, 16), np.float32)
    for c in range(CPG):
        em[c * M:(c + 1) * M, c] = 1.0

    in_maps = []
    for k in range(N_CORES):
        sl = mem_b[k * C_PER:(k + 1) * C_PER]                      # [125,11,1024]
        pad = np.zeros((NG * CPG, M, D), bf)
        pad[:C_PER] = sl
        grp = pad.reshape(NG, PG, D)
        full = np.zeros((NG, 128, D), bf)
        full[:, :PG] = grp
        membf = np.empty((64 + ROWS, D), bf)
        membf[:64] = img_b
        membf[64:] = full.reshape(ROWS, D)
        in_maps.append({"membf": membf, "mask": mask, "em": em})
    return in_maps


def _gather(results):
    logits = np.empty((B, C), np.float32)
    for k in range(N_CORES):
        o = results[k]["out"].reshape(16, NG, 64)[:CPG]            # [11, 12, 64]
        o = o.transpose(1, 0, 2).reshape(NG * CPG, 64)[:C_PER]     # [125, 64]
        logits[:, k * C_PER:(k + 1) * C_PER] = o.T
    return logits


def kernel(img_features, memorized_image_feat):
    from concourse.bass_utils import run_bass_kernel_spmd

    nc = _get_nc()
    in_maps = _prep_inputs(img_features, memorized_image_feat)
    res = run_bass_kernel_spmd(nc, in_maps, core_ids=list(range(N_CORES)))
    return _gather(res.results)
